# revision 20
# baseline (speedup 1.0000x reference)
"""Trainium2 Bass kernel for nn_AttentionBlock (BatchNorm + quirky-layout attention + proj).

Reference semantics (b=2, c=512, L=1024, num_heads=8):
  xn = batchnorm(x) (stats over batch+length per channel), gamma/beta affine
  qkv = w_qkv @ xn + b_qkv            (1x1 conv over channels)
  layout quirk: qkv -> (b, 3*nh, hd, L) -> (b, hd, L, 3*nh); split q,k,v
    => 64 attention "heads" (hd axis), feature dim 8 (nh axis), seq len 1024
  scores = softmax(scale * q @ k^T) over key axis, scale = (3*nh)**-0.5
  a = scores @ v ;  h[(d*64+hd), t] = a[hd, t, d] ;  out = x + w_proj @ h + b_proj

Sharding: 8 cores = 2 batches x 4 head-groups (16 heads each). Each core
computes BN (redundantly), its q/k/v projections, attention for its heads,
and a partial output projection over its 64 channels (padded to 512 with
zero weight rows). Host sums the 4 partials per batch (pure unshard of a
contraction-sharded output; residual x and b_proj are folded into one
core's input per batch).
"""
import numpy as np
import ml_dtypes

import concourse.bass as bass
import concourse.bacc as bacc
import concourse.mybir as mybir
import concourse.tile as tile
from concourse.bass_utils import run_bass_kernel_spmd

F32 = mybir.dt.float32
BF16 = mybir.dt.bfloat16

B, C, L = 2, 512, 1024
NH = 8          # feature dim of each attention head (from num_heads)
HD = 64         # number of attention heads (head_dim axis of the quirky layout)
HEADS_PER_CORE = 16
N_CORES = 8
EPS = 1e-5

_CACHE = {}


def _build_nc():
    """Build the single-NeuronCore program (SPMD across 8 cores)."""
    nc = bacc.Bacc(None, target_bir_lowering=False)

    # ---- DRAM I/O ----
    x2_d = nc.dram_tensor("x2", [C, 2 * L], BF16, kind="ExternalInput")       # [c, b*L]
    xres_d = nc.dram_tensor("xres", [C, L], F32, kind="ExternalInput")       # x_b + b_proj on lead cores, 0 else
    gamma_d = nc.dram_tensor("gamma", [C], F32, kind="ExternalInput")
    beta_d = nc.dram_tensor("beta", [C], F32, kind="ExternalInput")
    wq_d = nc.dram_tensor("wqT", [C, 512], BF16, kind="ExternalInput")       # [c, padded qch] (scale folded)
    wk_d = nc.dram_tensor("wkT", [C, 512], BF16, kind="ExternalInput")
    wv_d = nc.dram_tensor("wvT", [C, 128], BF16, kind="ExternalInput")       # [c, vch compact]
    bq_d = nc.dram_tensor("bq", [512], F32, kind="ExternalInput")            # padded, scale folded
    bk_d = nc.dram_tensor("bk", [512], F32, kind="ExternalInput")
    bv_d = nc.dram_tensor("bv", [128], F32, kind="ExternalInput")
    wp_d = nc.dram_tensor("wpT", [512, 512], BF16, kind="ExternalInput")     # [padded c, o]
    out_d = nc.dram_tensor("out", [C, L], F32, kind="ExternalOutput")
    rscr_d = nc.dram_tensor("rscr", [HEADS_PER_CORE, L], BF16)               # internal scratch (recip denoms)

    with tile.TileContext(nc) as tc:
        with (
            tc.tile_pool(name="singles", bufs=1) as singles,
            tc.tile_pool(name="wt", bufs=10) as wtp,
            tc.tile_pool(name="norm", bufs=3) as normp,
            tc.tile_pool(name="outp", bufs=3) as outp,
            tc.tile_pool(name="psb", bufs=3, space="PSUM") as psb,
            tc.tile_pool(name="psav", bufs=1, space="PSUM") as psav,
        ):
            # ---- load x and params ----
            xch = [[singles.tile([128, 512], BF16, name=f"xc{i}_{k}") for k in range(4)]
                   for i in range(4)]
            dma_engs = [nc.sync, nc.gpsimd, nc.scalar, nc.sync]
            for ct in range(4):
                for k in range(4):
                    dma_engs[(ct + k) % 4].dma_start(
                        xch[ct][k][:], x2_d[ct * 128:(ct + 1) * 128, k * 512:(k + 1) * 512])
            gam = singles.tile([128, 4], F32, name="gam")
            bet = singles.tile([128, 4], F32, name="bet")
            nc.sync.dma_start(gam[:], gamma_d.rearrange("(o p) -> p o", p=128))
            nc.sync.dma_start(bet[:], beta_d.rearrange("(o p) -> p o", p=128))
            wq = [singles.tile([128, 512], BF16, name=f"wq{i}") for i in range(4)]
            wk = [singles.tile([128, 512], BF16, name=f"wk{i}") for i in range(4)]
            wv = [singles.tile([128, 128], BF16, name=f"wv{i}") for i in range(4)]
            wp = [singles.tile([128, 512], BF16, name=f"wp{i}") for i in range(4)]
            for ct in range(4):
                nc.sync.dma_start(wq[ct][:], wq_d[ct * 128:(ct + 1) * 128, :])
                nc.gpsimd.dma_start(wk[ct][:], wk_d[ct * 128:(ct + 1) * 128, :])
                nc.sync.dma_start(wv[ct][:], wv_d[ct * 128:(ct + 1) * 128, :])
            bqt = singles.tile([128, 4], F32, name="bqt")
            bkt = singles.tile([128, 4], F32, name="bkt")
            nc.sync.dma_start(bqt[:], bq_d.rearrange("(o p) -> p o", p=128))
            nc.sync.dma_start(bkt[:], bk_d.rearrange("(o p) -> p o", p=128))
            bvb = singles.tile([128, 128], F32, name="bvb")  # bv broadcast across partitions
            nc.sync.dma_start(bvb[:], bass.AP(tensor=bv_d.tensor if hasattr(bv_d, "tensor") else bv_d,
                                              offset=0, ap=[[0, 128], [1, 128]]))
            xrt = [singles.tile([128, L], F32, name=f"xr{i}") for i in range(4)]
            epst = singles.tile([128, 1], F32, name="eps")
            nc.vector.memset(epst[:], EPS)

            # ---- BatchNorm stats + fused scale/shift ----
            # stats over the full (b, L) = 2048 free elements per channel
            mvs = []
            for ct in range(4):
                stats = normp.tile([128, 4, 6], F32, tag="bnstats")
                for k in range(4):
                    nc.vector.bn_stats(out=stats[:, k, :], in_=xch[ct][k][:])
                mv = singles.tile([128, 2], F32, name=f"mv{ct}")
                nc.vector.bn_aggr(out=mv[:], in_=stats[:])
                mvs.append(mv)
            sc = []  # per c-tile (scale, shift) [128,1]
            for ct in range(4):
                mv = mvs[ct]
                # rstd = 1/sqrt(var+eps) via bit-trick + 2 Newton iterations
                r_t = singles.tile([128, 1], F32, name=f"r{ct}")
                nc.vector.tensor_scalar(out=r_t[:], in0=mv[:, 1:2], scalar1=EPS,
                                        scalar2=None, op0=mybir.AluOpType.add)
                yi = singles.tile([128, 1], mybir.dt.int32, name=f"yi{ct}")
                with nc.allow_low_precision(reason="rsqrt seed bit trick"):
                    # yi = (2*0x5f3759df - bits(r)) >> 1  ==  0x5f3759df - (bits(r)>>1)
                    nc.vector.tensor_scalar(out=yi[:], in0=r_t.bitcast(mybir.dt.int32)[:],
                                            scalar1=-1, scalar2=2 * 0x5f3759df,
                                            op0=mybir.AluOpType.mult, op1=mybir.AluOpType.add)
                    nc.vector.tensor_scalar(out=yi[:], in0=yi[:], scalar1=1,
                                            scalar2=None,
                                            op0=mybir.AluOpType.logical_shift_right)
                rstd = singles.tile([128, 1], F32, name=f"rstd{ct}")
                yf = yi.bitcast(F32)
                for it in range(2):
                    t2 = singles.tile([128, 1], F32, name=f"t2_{ct}_{it}")
                    nc.vector.tensor_tensor(out=t2[:], in0=yf[:], in1=yf[:],
                                            op=mybir.AluOpType.mult)
                    nc.vector.tensor_tensor(out=t2[:], in0=t2[:], in1=r_t[:],
                                            op=mybir.AluOpType.mult)
                    nc.vector.tensor_scalar(out=t2[:], in0=t2[:], scalar1=-0.5,
                                            scalar2=1.5, op0=mybir.AluOpType.mult,
                                            op1=mybir.AluOpType.add)
                    dst = rstd if it == 1 else yi.bitcast(F32)
                    nc.vector.tensor_tensor(out=dst[:], in0=yf[:], in1=t2[:],
                                            op=mybir.AluOpType.mult)
                s_t = singles.tile([128, 1], F32, name=f"s{ct}")
                nc.vector.tensor_tensor(out=s_t[:], in0=rstd[:], in1=gam[:, ct:ct + 1],
                                        op=mybir.AluOpType.mult)
                tmp = singles.tile([128, 1], F32, name=f"tmp{ct}")
                nc.vector.tensor_tensor(out=tmp[:], in0=mv[:, 0:1], in1=s_t[:],
                                        op=mybir.AluOpType.mult)
                t_t = singles.tile([128, 1], F32, name=f"t{ct}")
                nc.vector.tensor_tensor(out=t_t[:], in0=bet[:, ct:ct + 1], in1=tmp[:],
                                        op=mybir.AluOpType.subtract)
                sc.append((s_t, t_t))

            # ---- xn for own batch (bf16) : xn = x*s + t ----
            # own-batch slice is provided via xres?? no: batch column range passed by host
            # host packs x2 so that cols [0:L] are ALWAYS the own batch (see host prep)
            xn = [singles.tile([128, L], BF16, name=f"xn{i}") for i in range(4)]
            for ct in range(4):
                s_t, t_t = sc[ct]
                for k in range(2):
                    nc.vector.tensor_scalar(out=xn[ct][:, k * 512:(k + 1) * 512],
                                            in0=xch[ct][k][:],
                                            scalar1=s_t[:], scalar2=t_t[:],
                                            op0=mybir.AluOpType.mult, op1=mybir.AluOpType.add)

            # ---- k/q projections (padded head-block layout) ----
            kT = [singles.tile([128, L], BF16, name=f"kT{i}") for i in range(4)]
            qT = [singles.tile([128, L], BF16, name=f"qT{i}") for i in range(4)]
            for which, wmat, bias_t, dst in (("k", wk, bkt, kT), ("q", wq, bqt, qT)):
                for mo in range(4):
                    ps = psb.tile([128, 1024], F32, tag="big")
                    for kt in range(4):
                        for nh_ in range(2):
                            nc.tensor.matmul(
                                ps[:, nh_ * 512:(nh_ + 1) * 512],
                                wmat[kt][:, mo * 128:(mo + 1) * 128],
                                xn[kt][:, nh_ * 512:(nh_ + 1) * 512],
                                start=(kt == 0), stop=(kt == 3))
                    nc.vector.tensor_scalar(out=dst[mo][:], in0=ps[:],
                                            scalar1=bias_t[:, mo:mo + 1],
                                            scalar2=None,
                                            op0=mybir.AluOpType.add)

            # ---- v projection (flipped: [t, vch]) + v_sb blocks ----
            # v_sb[fb]: [128 f, 16 heads, 32] = [v(8), one(1), zeros(23)]
            vsb = [singles.tile([128, 16, 32], BF16, name=f"vsb{i}") for i in range(8)]
            for fb in range(8):
                nc.gpsimd.memset(vsb[fb][:, :, 8:9], 1.0)
                nc.gpsimd.memset(vsb[fb][:, :, 9:32], 0.0)
            for tt in range(8):
                ps_full = psb.tile([128, 1024], F32, tag="big", name="psv")
                ps = ps_full[:, 0:128]
                for kt in range(4):
                    nc.tensor.matmul(ps[:], xn[kt][:, tt * 128:(tt + 1) * 128],
                                     wv[kt][:], start=(kt == 0), stop=(kt == 3))
                nc.vector.tensor_tensor(
                    out=vsb[tt][:, :, 0:8],
                    in0=ps.rearrange("p (h d) -> p h d", d=8),
                    in1=bvb.rearrange("p (h d) -> p h d", d=8),
                    op=mybir.AluOpType.add)

            # ---- attention: quad-major, f-block inner ----
            hout = [singles.tile([128, L], BF16, name=f"ho{i}") for i in range(4)]
            for qd in range(4):
                if qd == 1:
                    for ct in range(4):
                        nc.sync.dma_start(wp[ct][:], wp_d[ct * 128:(ct + 1) * 128, :])
                        nc.gpsimd.dma_start(xrt[ct][:], xres_d[ct * 128:(ct + 1) * 128, :])
                av = psav.tile([128, 1024], F32, tag="av")
                wt_q = {}

                def emit_score_mm(fb, j, ps, nh_):
                    nc.tensor.matmul(
                        ps[:, nh_ * 512:(nh_ + 1) * 512],
                        kT[qd][32 * j:32 * j + 32, fb * 128:(fb + 1) * 128],
                        qT[qd][32 * j:32 * j + 32, nh_ * 512:(nh_ + 1) * 512],
                        tile_position=(32 * j, 0))

                def emit_exp(fb, j, ps):
                    if (fb * 4 + j) % 16 in (1, 2, 4, 6, 9, 11, 14):
                        wti = wtp.tile([128, 1024], mybir.dt.int16, tag="wt")
                        with nc.allow_low_precision(reason="schraudolph exp approx, validated"):
                            nc.vector.tensor_scalar(
                                out=wti[:], in0=ps[:],
                                scalar1=184.66496, scalar2=16248.75,
                                op0=mybir.AluOpType.mult, op1=mybir.AluOpType.add)
                        wt = wti.bitcast(BF16)
                    else:
                        wt = wtp.tile([128, 1024], BF16, tag="wt")
                        nc.scalar.activation(out=wt[:], in_=ps[:],
                                             func=mybir.ActivationFunctionType.Exp)
                    wt_q.setdefault(fb, {})[j] = wt

                def emit_av_half(fb, nh_):
                    for j in range(4):
                        nc.tensor.matmul(
                            av[32 * j:32 * j + 32, nh_ * 512:(nh_ + 1) * 512],
                            vsb[fb][:, 4 * qd + j, :],
                            wt_q[fb][j][:, nh_ * 512:(nh_ + 1) * 512],
                            start=(fb == 0), stop=(fb == 7),
                            tile_position=(0, 32 * j))

                for fbx in range(9):
                    if fbx < 8:
                        pss = {j: psb.tile([128, 1024], F32, tag="big", name="ps_sc")
                               for j in range(3)}
                        for nh_ in range(2):
                            for j in range(3):
                                emit_score_mm(fbx, j, pss[j], nh_)
                        for j in range(3):
                            emit_exp(fbx, j, pss[j])
                    if fbx >= 1:
                        emit_av_half(fbx - 1, 0)
                        emit_av_half(fbx - 1, 1)
                        wt_q.pop(fbx - 1)
                    if fbx < 8:
                        ps3 = psb.tile([128, 1024], F32, tag="big", name="ps_sc")
                        for nh_ in range(2):
                            emit_score_mm(fbx, 3, ps3, nh_)
                        emit_exp(fbx, 3, ps3)
                # normalize quad
                a_sb = normp.tile([128, 1024], BF16, tag="asb")
                nc.vector.tensor_copy(a_sb[:], av[:])
                dt = normp.tile([32, 128], BF16, tag="dt")
                for j in range(4):
                    nc.sync.dma_start(
                        dt[8 * j:8 * j + 8, :],
                        a_sb[32 * j + 8:32 * j + 9, :].rearrange("p (s f) -> p s f", f=128))
                rt = normp.tile([32, 128], BF16, tag="rt")
                with nc.allow_low_precision(reason="bf16 softmax denom recip, validated 2e-4 rel err"):
                    nc.vector.reciprocal(out=rt[:], in_=dt[:])
                for j in range(4):
                    h = 4 * qd + j
                    nc.sync.dma_start(rscr_d[h, :].rearrange("(s f) -> s f", f=128),
                                      rt[8 * j:8 * j + 8, :])
                rb = normp.tile([128, 1024], BF16, tag="rb")
                for j in range(4):
                    h = 4 * qd + j
                    nc.sync.dma_start(
                        rb[32 * j:32 * j + 32, :],
                        bass.AP(tensor=rscr_d.tensor if hasattr(rscr_d, "tensor") else rscr_d,
                                offset=h * L, ap=[[0, 32], [1, L]]))
                nc.gpsimd.tensor_tensor(out=hout[qd][:], in0=a_sb[:], in1=rb[:],
                                         op=mybir.AluOpType.mult)

            # ---- output projection (partial over this core's channels) + residual ----
            for mo in range(4):
                for th in range(2):
                    ps_full = psb.tile([128, 1024], F32, tag="big", name="pso")
                    ps = ps_full[:, 0:512]
                    for qd in range(4):
                        nc.tensor.matmul(ps[:], wp[qd][:, mo * 128:(mo + 1) * 128],
                                         hout[qd][:, th * 512:(th + 1) * 512],
                                         start=(qd == 0), stop=(qd == 3))
                    ot = outp.tile([128, 512], F32, tag="ot")
                    nc.vector.tensor_tensor(out=ot[:], in0=ps[:],
                                            in1=xrt[mo][:, th * 512:(th + 1) * 512],
                                            op=mybir.AluOpType.add)
                    nc.sync.dma_start(out_d[mo * 128:(mo + 1) * 128, th * 512:(th + 1) * 512], ot[:])

    nc.compile()
    return nc


def _prep_inputs(x, gamma, beta, w_qkv, b_qkv, w_proj, b_proj, num_heads):
    """Shard and lay out inputs for the 8 cores."""
    nh = int(num_heads)
    hd = C // nh
    scale = (3 * nh) ** (-0.5)
    wq_full, wk_full, wv_full = w_qkv[0:C], w_qkv[C:2 * C], w_qkv[2 * C:3 * C]
    bq_full, bk_full, bv_full = b_qkv[0:C], b_qkv[C:2 * C], b_qkv[2 * C:3 * C]

    bf16 = ml_dtypes.bfloat16
    in_maps = []
    for core in range(N_CORES):
        bi = core // 4
        g = core % 4
        heads = list(range(HEADS_PER_CORE * g, HEADS_PER_CORE * (g + 1)))
        # x2: own batch first, other batch second (BN stats use both)
        x_own = x[bi]
        x_oth = x[1 - bi]
        x2 = np.concatenate([x_own, x_oth], axis=1).astype(bf16)

        # padded q/k weight layouts: [c, 512], col 32*jl + d = channel d*hd + h
        wqT = np.zeros((C, 512), np.float32)
        wkT = np.zeros((C, 512), np.float32)
        bq = np.zeros(512, np.float32)
        bk = np.zeros(512, np.float32)
        wvT = np.zeros((C, 128), np.float32)
        bv = np.zeros(128, np.float32)
        wpT = np.zeros((512, C), np.float32)
        for jl, h in enumerate(heads):
            for d in range(nh):
                ch = d * hd + h
                wqT[:, 32 * jl + d] = wq_full[ch] * scale
                wkT[:, 32 * jl + d] = wk_full[ch]
                bq[32 * jl + d] = bq_full[ch] * scale
                bk[32 * jl + d] = bk_full[ch]
                wvT[:, 8 * jl + d] = wv_full[ch]
                bv[8 * jl + d] = bv_full[ch]
                wpT[32 * jl + d, :] = w_proj[:, ch]

        xres = np.zeros((C, L), np.float32)
        if g == 0:
            xres = (x[bi] + b_proj[:, None]).astype(np.float32)

        in_maps.append({
            "x2": np.ascontiguousarray(x2),
            "xres": np.ascontiguousarray(xres),
            "gamma": np.ascontiguousarray(gamma.astype(np.float32)),
            "beta": np.ascontiguousarray(beta.astype(np.float32)),
            "wqT": np.ascontiguousarray(wqT.astype(bf16)),
            "wkT": np.ascontiguousarray(wkT.astype(bf16)),
            "wvT": np.ascontiguousarray(wvT.astype(bf16)),
            "bq": bq, "bk": bk, "bv": bv,
            "wpT": np.ascontiguousarray(wpT.astype(bf16)),
        })
    return in_maps


def kernel(x, gamma, beta, w_qkv, b_qkv, w_proj, b_proj, num_heads, _trace=False):
    x = np.asarray(x, dtype=np.float32)
    gamma = np.asarray(gamma, np.float32)
    beta = np.asarray(beta, np.float32)
    w_qkv = np.asarray(w_qkv, np.float32)
    b_qkv = np.asarray(b_qkv, np.float32)
    w_proj = np.asarray(w_proj, np.float32)
    b_proj = np.asarray(b_proj, np.float32)

    if "nc" not in _CACHE:
        _CACHE["nc"] = _build_nc()
    nc = _CACHE["nc"]
    in_maps = _prep_inputs(x, gamma, beta, w_qkv, b_qkv, w_proj, b_proj, num_heads)
    res = run_bass_kernel_spmd(nc, in_maps, core_ids=list(range(N_CORES)), trace=_trace)
    _CACHE["last_result"] = res

    out = np.zeros((B, C, L), np.float32)
    for bi in range(B):
        acc = np.zeros((C, L), np.float32)
        for g in range(4):
            acc += np.asarray(res.results[bi * 4 + g]["out"])
        out[bi] = acc
    return out
